# revision 1
# baseline (speedup 1.0000x reference)
"""Distributed causal attention head on 8 TRN2 NeuronCores.

Problem: B=4, S=4096, D_in=512, D_out=64 causal attention
  K/V/Q = X @ W; scores = Q@K^T (causal, /sqrt(64)); Z = softmax(scores)@V

Sharding: core c = 2*b + h handles batch b, seq-half h.
q-rows are interleaved at 128-row-block granularity (core h owns global
q-blocks {2j+h}), which makes the causal block schedule IDENTICAL on all
cores (SPMD-safe) and balances FLOPs exactly.  Every core loads the full
(transposed) K/V inputs of its batch and projects them locally.

The whole kernel is interleaved at q-chunk granularity so the PE never
idles >3.4us (HAM stays warm) and compute overlaps the input DMA stream:
for each chunk c: DMA xq[c], xk/xv[2c:2c+2] (separate small tiles ->
precise Tile deps), project Q/K/V for just those columns, PE-transpose
the new V blocks, then run the chunk's attention.  Matmul inputs bf16,
psum/softmax f32.  Scores are computed transposed ST[k,q] with KpT
parity-packed so score matmuls run as row-tiled K=64 PAIRS; exp on ACT
in groups of 3 kblocks (scale=1/8 folded, no max-subtraction:
|scores/8| < ~1.5); AV matmuls accumulate Z^T in PSUM with a
ones-column in Vp giving the softmax denominator for free; Z^T is
PE-transposed back to q-major and normalized with a per-partition
reciprocal + tensor_scalar_mul; output is q-major [2048, 64] f32.
"""

import numpy as np
import ml_dtypes

import concourse.bass as bass
import concourse.bacc as bacc
import concourse.mybir as mybir
import concourse.tile as tile

B, S, D, E = 4, 4096, 512, 64
PB = 128                      # partition block
NKB = S // PB                 # 32 k-blocks (global)
NLQ = NKB // 2                # 16 local q-blocks per core
NCH = 4                       # q-chunks of 512 per core
CHW = 512                     # q-chunk width
ND = D // PB                  # 4 d-slices
GRP = 2                       # kblocks per exp group
LAG = 4                       # ST->AV software pipeline depth (groups)
BF16 = mybir.dt.bfloat16
F32 = mybir.dt.float32
NPBF16 = ml_dtypes.bfloat16


def kparity(kb):
    """kblock -> (partition base, chunk idx, col) in parity-packed kpT."""
    return 64 * (kb % 2), kb // 4, PB * ((kb // 2) % 2)


def build_nc():
    nc = bacc.Bacc(None)

    xq_d = nc.declare_dram_parameter("xq", [D, S // 2], BF16, isOutput=False)
    xk_d = nc.declare_dram_parameter("xk", [D, S], BF16, isOutput=False)
    xv_d = nc.declare_dram_parameter("xv", [D, S], BF16, isOutput=False)
    wq_d = nc.declare_dram_parameter("wq", [D, E], BF16, isOutput=False)
    wk_d = nc.declare_dram_parameter("wk", [D, E], BF16, isOutput=False)
    wv_d = nc.declare_dram_parameter("wv", [D, E], BF16, isOutput=False)
    cm_d = nc.declare_dram_parameter("cmask", [8, PB, CHW], BF16, isOutput=False)
    id_d = nc.declare_dram_parameter("ident", [PB, PB], F32, isOutput=False)
    out_d = nc.declare_dram_parameter("out", [S // 2, E], F32, isOutput=True)

    with tile.TileContext(nc) as tc:
        with tc.tile_pool(name="persist", bufs=1) as pp, \
             tc.tile_pool(name="st_ps", bufs=2, space="PSUM") as stp, \
             tc.tile_pool(name="pj_ps", bufs=2, space="PSUM") as pjp, \
             tc.tile_pool(name="zt_ps", bufs=2, space="PSUM") as ztp, \
             tc.tile_pool(name="work", bufs=2 * LAG + 2) as wp, \
             tc.tile_pool(name="osb", bufs=3) as op:
            # ---- persistent SBUF tiles ----
            wq_sb = pp.tile([PB, ND * E], BF16, name="wq_sb", tag="wq_sb")
            wk_sb = pp.tile([PB, ND * E], BF16, name="wk_sb", tag="wk_sb")
            wv_sb = pp.tile([PB, ND * E], BF16, name="wv_sb", tag="wv_sb")
            mk_sb = pp.tile([PB, 8 * CHW], BF16, name="mk_sb", tag="mk_sb")
            idf_sb = pp.tile([PB, PB], F32, name="idf_sb", tag="idf_sb")
            idb_sb = pp.tile([PB, PB], BF16, name="idb_sb", tag="idb_sb")
            # per-half input tiles (one DMA each -> precise, cheap deps)
            xq_sb = [[pp.tile([PB, 2 * CHW], BF16, name=f"xq{d}_{g}", tag=f"xq{d}_{g}")
                      for g in range(2)] for d in range(ND)]
            xk_sb = [[pp.tile([PB, 4 * CHW], BF16, name=f"xk{d}_{g}", tag=f"xk{d}_{g}")
                      for g in range(2)] for d in range(ND)]
            xv_sb = [[pp.tile([PB, 4 * CHW], BF16, name=f"xv{d}_{g}", tag=f"xv{d}_{g}")
                      for g in range(2)] for d in range(ND)]
            # projected tensors, chunked
            qpT = [pp.tile([PB, CHW], BF16, name=f"qpT{c}", tag=f"qpT{c}")
                   for c in range(NCH)]                    # dup both halves
            kpT = [pp.tile([PB, 2 * PB], BF16, name=f"kpT{c}", tag=f"kpT{c}")
                   for c in range(2 * NCH)]                # parity-packed
            vpT = [pp.tile([E, CHW], BF16, name=f"vpT{c}", tag=f"vpT{c}")
                   for c in range(2 * NCH)]
            vp = [pp.tile([PB, E + 1], BF16, name=f"vp{s}", tag=f"vp{s}")
                  for s in range(NKB)]

            # ---- constant DMAs (one each, on the fast sync queue, first) ----
            for w_d, w_sb in ((wq_d, wq_sb), (wk_d, wk_sb), (wv_d, wv_sb)):
                nc.sync.dma_start(
                    out=w_sb[:].rearrange("p (d e) -> p d e", e=E),
                    in_=w_d.rearrange("(d p) e -> p d e", p=PB))
            for s in range(NKB):
                nc.vector.memset(vp[s][:], 1.0)   # ones column prefill

            def dma_inputs(g):
                """Issue input DMAs for half g: xq cols, xk/xv cols."""
                for d in range(ND):
                    nc.sync.dma_start(
                        out=xq_sb[d][g][:],
                        in_=xq_d[PB * d:PB * (d + 1), 2 * CHW * g:2 * CHW * (g + 1)])
                for d in range(ND):
                    nc.sync.dma_start(
                        out=xk_sb[d][g][:],
                        in_=xk_d[PB * d:PB * (d + 1), 4 * CHW * g:4 * CHW * (g + 1)])
                if g == 0:
                    nc.gpsimd.dma_start(out=idf_sb[:], in_=id_d[:])
                    nc.vector.tensor_copy(idb_sb[:], idf_sb[:])
                    nc.gpsimd.dma_start(
                        out=mk_sb[:].rearrange("p (m q) -> p m q", q=CHW),
                        in_=cm_d.rearrange("m p q -> p m q"))
                for d in range(ND):
                    nc.sync.dma_start(
                        out=xv_sb[d][g][:],
                        in_=xv_d[PB * d:PB * (d + 1), 4 * CHW * g:4 * CHW * (g + 1)])

            def vtrans(s):
                """PE-transpose one projected-V block to k-major + copy out."""
                vproj(s // 4)
                vt_ps = pjp.tile([PB, E], BF16, tag="pj")
                nc.tensor.transpose(vt_ps[:], vpT[s // 4][:, PB * (s % 4):PB * (s % 4 + 1)],
                                    idb_sb[0:E, 0:E])
                nc.vector.tensor_copy(vp[s][:, 0:E], vt_ps[:])

            def project(c):
                """Project Q chunk c and K/V chunks 2c, 2c+1 (V transposes
                are emitted later, interleaved between ST groups)."""
                g = c // 2
                qof = CHW * (c % 2)
                qp_ps = pjp.tile([E, CHW], F32, tag="pj")
                for d in range(ND):
                    nc.tensor.matmul(qp_ps[:], wq_sb[:, E * d:E * (d + 1)],
                                     xq_sb[d][g][:, qof:qof + CHW],
                                     start=(d == 0), stop=(d == ND - 1))
                nc.vector.tensor_copy(qpT[c][0:E, :], qp_ps[:])
                nc.scalar.copy(qpT[c][E:2 * E, :], qp_ps[:])
                for kc in (2 * c, 2 * c + 1):
                    kof = CHW * (kc % 4)
                    kp_ps = pjp.tile([E, CHW], F32, tag="pj")
                    for d in range(ND):
                        nc.tensor.matmul(kp_ps[:], wk_sb[:, E * d:E * (d + 1)],
                                         xk_sb[d][g][:, kof:kof + CHW],
                                         start=(d == 0), stop=(d == ND - 1))
                    for j in range(4):
                        kb = 4 * kc + j
                        pb, kch, col = kparity(kb)
                        assert kch == kc
                        nc.vector.tensor_copy(kpT[kc][pb:pb + E, col:col + PB],
                                              kp_ps[:, PB * j:PB * (j + 1)])
            vproj_done = set()

            def vproj(kc):
                """Lazily project V chunk kc (called at first vtrans use)."""
                if kc in vproj_done:
                    return
                vproj_done.add(kc)
                kof = CHW * (kc % 4)
                vq_ps = pjp.tile([E, CHW], F32, tag="pj")
                for d in range(ND):
                    nc.tensor.matmul(vq_ps[:], wv_sb[:, E * d:E * (d + 1)],
                                     xv_sb[d][kc // 4][:, kof:kof + CHW],
                                     start=(d == 0), stop=(d == ND - 1))
                nc.vector.tensor_copy(vpT[kc][:], vq_ps[:])

            def st_mm(st_ps, ji, kb, c):
                pb, kch, col = kparity(kb)
                nc.tensor.matmul(st_ps[:, CHW * ji:CHW * (ji + 1)],
                                 kpT[kch][pb:pb + E, col:col + PB],
                                 qpT[c][pb:pb + E, :],
                                 start=True, stop=True, tile_position=(pb, 0))

            # prologue: first half's DMA + first chunk's projections
            dma_inputs(0)
            project(0)

            norm_pend = None
            for c in range(NCH):
                nkb = 8 * c + 8
                zt_ps = ztp.tile([E + 1, CHW], F32, tag="zt")
                korder = list(range(0, nkb))
                groups = [korder[i:i + GRP] for i in range(0, nkb, GRP)]
                pend = []
                drain_state = {"n": 0}

                def drain_avs(p_et, p_kbs, nkb=nkb, zt_ps=zt_ps, c=c, ds=drain_state):
                    for kb in p_kbs:      # late vtrans, spread across groups
                        if kb >= 8 * c:
                            vtrans(kb)
                    for ji, kb in enumerate(p_kbs):
                        nc.tensor.matmul(
                            zt_ps[:], vp[kb][:],
                            p_et[:, CHW * ji:CHW * (ji + 1)],
                            start=(ds["n"] == 0),
                            stop=(ds["n"] == nkb - 1),
                            skip_group_check=True)
                        ds["n"] += 1

                if c == 0:
                    dma_inputs(1)   # stream second half's inputs early
                for gi, kbs in enumerate(groups):
                    gw = len(kbs) * CHW
                    st_ps = stp.tile([PB, GRP * CHW], F32, tag="st")
                    # pair of consecutive kblocks -> concurrent row-tiled MMs
                    if len(kbs) >= 2:
                        st_mm(st_ps, 0, kbs[0], c)
                        st_mm(st_ps, 1, kbs[1], c)
                        rest = range(2, len(kbs))
                    else:
                        rest = range(len(kbs))
                    for ji in rest:
                        st_mm(st_ps, ji, kbs[ji], c)
                    if len(pend) > LAG - 1:
                        drain_avs(*pend.pop(0))
                    et_sb = wp.tile([PB, GRP * CHW], BF16, tag="et")
                    nc.scalar.activation(
                        et_sb[:, :gw], st_ps[:, :gw],
                        mybir.ActivationFunctionType.Exp, scale=0.125)
                    for ji, kb in enumerate(kbs):
                        m = kb - 8 * c
                        if m >= 0:
                            nc.vector.tensor_mul(
                                et_sb[:, CHW * ji:CHW * (ji + 1)],
                                et_sb[:, CHW * ji:CHW * (ji + 1)],
                                mk_sb[:, CHW * m:CHW * (m + 1)])
                    pend.append((et_sb, kbs))
                for p in pend:
                    drain_avs(*p)
                zs_sb = wp.tile([E + 1, CHW], F32, tag="zs")
                nc.vector.tensor_copy(zs_sb[:], zt_ps[:])
                # project next chunk while exp/AV tail of this chunk drains
                if c + 1 < NCH:
                    project(c + 1)
                # normalize via transpose (denominator = col E)
                for j in range(4):
                    zn_ps = ztp.tile([PB, E + 1], F32, tag="zt")
                    nc.tensor.transpose(zn_ps[:], zs_sb[:, PB * j:PB * (j + 1)],
                                        idf_sb[0:E + 1, 0:E + 1])
                    rc_sb = wp.tile([PB, 1], F32, tag="rc")
                    nc.vector.reciprocal(rc_sb[:], zn_ps[:, E:E + 1])
                    o_sb = op.tile([PB, E], F32, tag="osb")
                    nc.vector.tensor_scalar_mul(o_sb[:], zn_ps[:, 0:E], rc_sb[:])
                    q0 = PB * (4 * c + j)
                    nc.gpsimd.dma_start(out=out_d[q0:q0 + PB, :], in_=o_sb[:])
    nc.finalize()
    return nc


def make_core_inputs(key_np, value_np, query_np, Wk, Wv, Wq):
    """Host-side sharding: returns in_maps list of 8 dicts."""
    bf = lambda a: np.ascontiguousarray(a).astype(NPBF16)
    in_maps = []
    for c in range(8):
        b, h = c // 2, c % 2
        qrows = np.concatenate(
            [np.arange(PB * (2 * j + h), PB * (2 * j + h) + PB) for j in range(NLQ)])
        # causal masks: mask m applies to kblock kb = 8c+m of every chunk;
        # section jj (q sub-block) has global q-block g = 8c+2jj+h,
        # class = m - 2jj - h: <0 keep, ==0 triangular, >0 zero.
        cmask = np.zeros((8, PB, CHW), dtype=np.float32)
        ki = np.arange(PB)[:, None]
        qi = np.arange(PB)[None, :]
        tri = (ki <= qi).astype(np.float32)
        for m in range(8):
            for jj in range(4):
                cls = m - 2 * jj - h
                blk = np.ones((PB, PB), np.float32) if cls < 0 else (
                    tri if cls == 0 else np.zeros((PB, PB), np.float32))
                cmask[m][:, PB * jj:PB * (jj + 1)] = blk
        in_maps.append({
            "xq": bf(query_np[b][qrows].T),
            "xk": bf(key_np[b].T),
            "xv": bf(value_np[b].T),
            "wq": bf(Wq), "wk": bf(Wk), "wv": bf(Wv),
            "cmask": bf(cmask),
            "ident": np.eye(PB, dtype=np.float32),
        })
    return in_maps


def assemble_output(results):
    """results: list of 8 dicts with 'out' [2048, 64] f32 -> Z [B,S,E]."""
    Z = np.zeros((B, S, E), dtype=np.float32)
    for c in range(8):
        b, h = c // 2, c % 2
        o = results[c]["out"]  # [2048, E] q-major
        for j in range(NLQ):
            g = 2 * j + h
            Z[b, PB * g:PB * (g + 1), :] = o[PB * j:PB * (j + 1), :]
    return Z


def kernel(key_inputs, value_inputs, query_inputs, Wk, Wv, Wq):
    from concourse.bass_utils import run_bass_kernel_spmd
    nc = build_nc()
    in_maps = make_core_inputs(np.asarray(key_inputs), np.asarray(value_inputs),
                               np.asarray(query_inputs), np.asarray(Wk),
                               np.asarray(Wv), np.asarray(Wq))
    res = run_bass_kernel_spmd(nc, in_maps, core_ids=list(range(8)))
    return assemble_output(res.results)



# revision 2
# speedup vs baseline: 1.0723x; 1.0723x over previous
"""Distributed causal attention head on 8 TRN2 NeuronCores.

Problem: B=4, S=4096, D_in=512, D_out=64 causal attention
  K/V/Q = X @ W; scores = Q@K^T (causal, /sqrt(64)); Z = softmax(scores)@V

Sharding: core c = 2*b + h handles batch b, seq-half h.
q-rows are interleaved at 128-row-block granularity (core h owns global
q-blocks {2j+h}), which makes the causal block schedule IDENTICAL on all
cores (SPMD-safe) and balances FLOPs exactly.  Every core loads the full
(transposed) K/V inputs of its batch and projects them locally.

v2 structure (vs the first working version):
 - Causal trapezoid trimming: for the 8 "new" kblocks of each q-chunk,
   scores/exp/AV are computed only on q-columns at-or-below the causal
   boundary (conservatively over both h parities so the program stays
   SPMD-identical; per-core mask8 input handles the h-dependent part).
   The mask shrinks from full [128,512] multiplies to one [128,2x128]
   multiply per diagonal group.
 - K and V projections run as column-tiled matmul PAIRS (tile_position
   (0,0)/(0,64)) so the 64-wide stationaries fill the whole PE array.
 - The PE is pre-warmed with ~34 dummy matmuls on a zero tile during the
   input-DMA wait so HAM un-throttles right as real work starts; the ACT
   exp table is pre-loaded by a dummy activation at t=0.
 - V-projection lands in PSUM partitions 64..127 and is staged/transposed
   entirely in the upper partition half (id2 identity lives there), so no
   downward cross-partition copies are needed.
 - Host permutes K/V columns within each 512-chunk ([j0 j2 j1 j3]) so the
   parity-packed kpT copies are two contiguous [64,256] copies.
 - Outputs are written per-chunk as one DMA; the last chunk goes on the
   fast sync (HWDGE) queue to shorten the tail.
Matmul inputs bf16, psum/softmax f32; exp with scale=1/8 folded, no
max-subtraction (|scores/8| < ~1.5); AV accumulates Z^T in PSUM with a
ones-column in Vp giving the softmax denominator for free.
"""

import numpy as np
import ml_dtypes

import concourse.bass as bass
import concourse.bacc as bacc
import concourse.mybir as mybir
import concourse.tile as tile

B, S, D, E = 4, 4096, 512, 64
PB = 128                      # partition block
NKB = S // PB                 # 32 k-blocks (global)
NLQ = NKB // 2                # 16 local q-blocks per core
NCH = 4                       # q-chunks of 512 per core
CHW = 512                     # q-chunk width
ND = D // PB                  # 4 d-slices
GRP = 2                       # kblocks per exp group
LAG = 4                       # ST->AV software pipeline depth (groups)
NWARM = 34                    # PE warmup matmuls
BF16 = mybir.dt.bfloat16
F32 = mybir.dt.float32
NPBF16 = ml_dtypes.bfloat16

# conservative (h-independent) causal column trim for new-kblock m=0..7:
# q-columns [C0TAB[m] : 512] of the chunk participate; the [128] section
# at C0TAB[m] gets the per-core mask8[m] multiply.
C0TAB = [0, 0, 128, 128, 256, 256, 384, 384]
# host K/V column permutation within each 512 chunk: [j0 j2 j1 j3]
KVPERM = [0, 2, 1, 3]         # block j of kc sits at slot KVPERM.index(j)
VSLOT = [0, 2, 1, 3]          # slot of block j: j=0->0, 1->2, 2->1, 3->3


def kparity(kb):
    """kblock -> (partition base, chunk idx, col) in parity-packed kpT."""
    return 64 * (kb % 2), kb // 4, PB * ((kb // 2) % 2)


def build_nc():
    nc = bacc.Bacc(None)

    xq_d = nc.declare_dram_parameter("xq", [D, S // 2], BF16, isOutput=False)
    xk_d = nc.declare_dram_parameter("xk", [D, S], BF16, isOutput=False)
    xv_d = nc.declare_dram_parameter("xv", [D, S], BF16, isOutput=False)
    wq_d = nc.declare_dram_parameter("wq", [D, E], BF16, isOutput=False)
    wk_d = nc.declare_dram_parameter("wk", [D, E], BF16, isOutput=False)
    wv_d = nc.declare_dram_parameter("wv", [D, E], BF16, isOutput=False)
    mk_d = nc.declare_dram_parameter("mask8", [8, PB, PB], BF16, isOutput=False)
    id_d = nc.declare_dram_parameter("ident", [PB, PB], F32, isOutput=False)
    id2_d = nc.declare_dram_parameter("ident2", [PB, E], BF16, isOutput=False)
    out_d = nc.declare_dram_parameter("out", [S // 2, E], F32, isOutput=True)

    with tile.TileContext(nc) as tc:
        with tc.tile_pool(name="persist", bufs=1) as pp, \
             tc.tile_pool(name="st_ps", bufs=2, space="PSUM") as stp, \
             tc.tile_pool(name="pj_ps", bufs=2, space="PSUM") as pjp, \
             tc.tile_pool(name="zt_ps", bufs=2, space="PSUM") as ztp, \
             tc.tile_pool(name="work", bufs=2 * LAG + 2) as wp, \
             tc.tile_pool(name="osb", bufs=2) as op:
            # ---- persistent SBUF tiles ----
            wq_sb = pp.tile([PB, ND * E], BF16, name="wq_sb", tag="wq_sb")
            wk_sb = pp.tile([PB, ND * E], BF16, name="wk_sb", tag="wk_sb")
            wv_sb = pp.tile([PB, ND * E], BF16, name="wv_sb", tag="wv_sb")
            mk_sb = pp.tile([PB, 8 * PB], BF16, name="mk_sb", tag="mk_sb")
            idf_sb = pp.tile([PB, PB], F32, name="idf_sb", tag="idf_sb")
            id2_sb = pp.tile([PB, E], BF16, name="id2_sb", tag="id2_sb")
            zpad = pp.tile([PB, PB], BF16, name="zpad", tag="zpad")
            aw_in = pp.tile([PB, 8], F32, name="aw_in", tag="aw_in")
            aw_out = pp.tile([PB, 8], F32, name="aw_out", tag="aw_out")
            # per-half input tiles (one DMA each -> precise, cheap deps);
            # half 0 of xk/xv is split in two for a faster prologue.
            xq_sb = [[pp.tile([PB, 2 * CHW], BF16, name=f"xq{d}_{g}", tag=f"xq{d}_{g}")
                      for g in range(2)] for d in range(ND)]
            xk_sb = [[pp.tile([PB, 4 * CHW], BF16, name=f"xk{d}_{g}", tag=f"xk{d}_{g}")
                      for g in range(2)] for d in range(ND)]
            xv_sb = [[pp.tile([PB, 4 * CHW], BF16, name=f"xv{d}_{g}", tag=f"xv{d}_{g}")
                      for g in range(2)] for d in range(ND)]
            # projected tensors, chunked
            qpT = [pp.tile([PB, CHW], BF16, name=f"qpT{c}", tag=f"qpT{c}")
                   for c in range(NCH)]                    # dup both halves
            kpT = [pp.tile([PB, 2 * PB], BF16, name=f"kpT{c}", tag=f"kpT{c}")
                   for c in range(2 * NCH)]                # parity-packed
            vpT = [pp.tile([PB, CHW], BF16, name=f"vpT{c}", tag=f"vpT{c}")
                   for c in range(2 * NCH)]                # upper half used
            vp = [pp.tile([PB, E + 1], BF16, name=f"vp{s}", tag=f"vp{s}")
                  for s in range(NKB)]

            # ---- t=0: ACT table preload + PE warmup on a zero tile ----
            nc.vector.memset(zpad[:], 0.0)
            nc.vector.memset(aw_in[:], 0.0)
            nc.scalar.activation(aw_out[:], aw_in[:],
                                 mybir.ActivationFunctionType.Exp, scale=0.125)
            for i in range(NWARM):
                wu = pjp.tile([PB, PB], F32, tag="pj")
                nc.tensor.matmul(wu[:], zpad[:], zpad[:], start=True, stop=True)
            for s in range(NKB):
                nc.gpsimd.memset(vp[s][:], 1.0)   # ones column prefill

            # ---- constant DMAs (one each, on the fast sync queue, first) ----
            for w_d, w_sb in ((wq_d, wq_sb), (wk_d, wk_sb), (wv_d, wv_sb)):
                nc.sync.dma_start(
                    out=w_sb[:].rearrange("p (d e) -> p d e", e=E),
                    in_=w_d.rearrange("(d p) e -> p d e", p=PB))

            def dma_inputs(g):
                """Issue input DMAs for half g: xq cols, then xk/xv cols.
                For half 0, xk goes first (chunk-0 ST path) and xv rides the
                scalar HWDGE queue so its descgen overlaps sync's."""
                for d in range(ND):
                    nc.sync.dma_start(
                        out=xq_sb[d][g][:],
                        in_=xq_d[PB * d:PB * (d + 1), 2 * CHW * g:2 * CHW * (g + 1)])
                for d in range(ND):
                    nc.sync.dma_start(
                        out=xk_sb[d][g][:],
                        in_=xk_d[PB * d:PB * (d + 1), 4 * CHW * g:4 * CHW * (g + 1)])
                if g == 0:
                    nc.gpsimd.dma_start(out=idf_sb[:], in_=id_d[:])
                    nc.gpsimd.dma_start(out=id2_sb[:], in_=id2_d[:])
                    nc.gpsimd.dma_start(
                        out=mk_sb[:].rearrange("p (m q) -> p m q", q=PB),
                        in_=mk_d.rearrange("m p q -> p m q"))
                for d in range(ND):
                    eng = nc.scalar if g == 0 else nc.sync
                    eng.dma_start(
                        out=xv_sb[d][g][:],
                        in_=xv_d[PB * d:PB * (d + 1), 4 * CHW * g:4 * CHW * (g + 1)])

            def vtrans(s):
                """PE-transpose one projected-V block (upper-half staging)
                to k-major + copy out to vp."""
                kc, j = s // 4, s % 4
                slot = VSLOT[j]
                vt_ps = pjp.tile([PB, E], BF16, tag="pj")
                nc.tensor.transpose(vt_ps[:],
                                    vpT[kc][E:2 * E, PB * slot:PB * (slot + 1)],
                                    id2_sb[E:2 * E, 0:E])
                nc.vector.tensor_copy(vp[s][:, 0:E], vt_ps[:])

            def project(c):
                """Project Q chunk c (solo) and K/V chunks 2c, 2c+1 as
                column-tiled pairs (K in partitions 0:64, V in 64:128)."""
                g = c // 2
                qof = CHW * (c % 2)
                qp_ps = pjp.tile([E, CHW], F32, tag="pj")
                for d in range(ND):
                    nc.tensor.matmul(qp_ps[:], wq_sb[:, E * d:E * (d + 1)],
                                     xq_sb[d][g][:, qof:qof + CHW],
                                     start=(d == 0), stop=(d == ND - 1))
                nc.vector.tensor_copy(qpT[c][0:E, :], qp_ps[:])
                nc.vector.tensor_copy(qpT[c][E:2 * E, :], qpT[c][0:E, :])
                for kc in (2 * c, 2 * c + 1):
                    kof = CHW * (kc % 4)
                    kv_ps = pjp.tile([PB, CHW], F32, tag="pj")
                    for d in range(ND):
                        nc.tensor.matmul(kv_ps[0:E, :], wk_sb[:, E * d:E * (d + 1)],
                                         xk_sb[d][g][:, kof:kof + CHW],
                                         start=(d == 0), stop=(d == ND - 1),
                                         skip_group_check=True)
                        nc.tensor.matmul(kv_ps[E:PB, :], wv_sb[:, E * d:E * (d + 1)],
                                         xv_sb[d][g][:, kof:kof + CHW],
                                         start=(d == 0), stop=(d == ND - 1),
                                         skip_group_check=True)
                    # host-permuted cols: [j0 j2 | j1 j3] -> two clean copies
                    nc.vector.tensor_copy(kpT[kc][0:E, :], kv_ps[0:E, 0:2 * PB])
                    nc.vector.tensor_copy(kpT[kc][E:PB, :], kv_ps[0:E, 2 * PB:4 * PB])
                    nc.vector.tensor_copy(vpT[kc][E:PB, :], kv_ps[E:PB, :])

            def st_mm(st_ps, ji, kb, c, c0):
                pb, kch, col = kparity(kb)
                nc.tensor.matmul(st_ps[:, CHW * ji + c0:CHW * (ji + 1)],
                                 kpT[kch][pb:pb + E, col:col + PB],
                                 qpT[c][pb:pb + E, c0:CHW],
                                 start=True, stop=True, tile_position=(pb, 0))

            # prologue: first half's DMA + first chunk's projections
            dma_inputs(0)
            project(0)

            for c in range(NCH):
                nkb = 8 * c + 8
                zt_ps = ztp.tile([E + 1, CHW], F32, tag="zt")
                groups = [list(range(i, i + GRP)) for i in range(0, nkb, GRP)]
                pend = []
                drain_state = {"n": 0, "tot": sum(
                    2 if kb < 8 * c else 2 for kb in range(0, nkb, 2))}
                nav = nkb  # number of AV matmuls this chunk

                def drain_avs(p_et, p_kbs, zt_ps=zt_ps, c=c, nav=nav,
                              ds=drain_state):
                    for kb in p_kbs:      # late vtrans, spread across groups
                        if kb >= 8 * c:
                            vtrans(kb)
                    for ji, kb in enumerate(p_kbs):
                        c0 = C0TAB[kb - 8 * c] if kb >= 8 * c else 0
                        nc.tensor.matmul(
                            zt_ps[:, c0:CHW], vp[kb][:],
                            p_et[:, CHW * ji + c0:CHW * (ji + 1)],
                            start=(ds["n"] == 0),
                            stop=(ds["n"] == nav - 1),
                            skip_group_check=True)
                        ds["n"] += 1

                if c == 0:
                    dma_inputs(1)   # stream second half's inputs early
                for gi, kbs in enumerate(groups):
                    diag = kbs[0] >= 8 * c
                    c0 = C0TAB[kbs[0] - 8 * c] if diag else 0
                    st_ps = stp.tile([PB, GRP * CHW], F32, tag="st")
                    st_mm(st_ps, 0, kbs[0], c, c0)
                    st_mm(st_ps, 1, kbs[1], c, c0)
                    if len(pend) > LAG - 1:
                        drain_avs(*pend.pop(0))
                    et_sb = wp.tile([PB, GRP * CHW], BF16, tag="et")
                    if c0 == 0:
                        nc.scalar.activation(
                            et_sb[:], st_ps[:],
                            mybir.ActivationFunctionType.Exp, scale=0.125)
                    else:
                        nc.scalar.activation(
                            et_sb[:].rearrange("p (u q) -> p u q", q=CHW)[:, :, c0:CHW],
                            st_ps[:].rearrange("p (u q) -> p u q", q=CHW)[:, :, c0:CHW],
                            mybir.ActivationFunctionType.Exp, scale=0.125)
                    if diag:
                        m0 = kbs[0] - 8 * c
                        nc.gpsimd.tensor_mul(
                            et_sb[:].rearrange("p (u q) -> p u q", q=CHW)[:, :, c0:c0 + PB],
                            et_sb[:].rearrange("p (u q) -> p u q", q=CHW)[:, :, c0:c0 + PB],
                            mk_sb[:, PB * m0:PB * (m0 + 2)].rearrange(
                                "p (u q) -> p u q", q=PB))
                    pend.append((et_sb, kbs))
                for p in pend:
                    drain_avs(*p)
                zs_sb = wp.tile([E + 1, CHW], F32, tag="zs")
                nc.vector.tensor_copy(zs_sb[:], zt_ps[:])
                # project next chunk while exp/AV tail of this chunk drains
                if c + 1 < NCH:
                    project(c + 1)
                # normalize via transpose (denominator = col E)
                o_sb = op.tile([PB, NCH * E], F32, tag="osb")
                for j in range(4):
                    zn_ps = ztp.tile([PB, E + 1], F32, tag="zt")
                    nc.tensor.transpose(zn_ps[:], zs_sb[:, PB * j:PB * (j + 1)],
                                        idf_sb[0:E + 1, 0:E + 1])
                    rc_sb = wp.tile([PB, 1], F32, tag="rc")
                    nc.vector.reciprocal(rc_sb[:], zn_ps[:, E:E + 1])
                    nc.vector.tensor_scalar_mul(o_sb[:, E * j:E * (j + 1)],
                                                zn_ps[:, 0:E], rc_sb[:])
                q0 = CHW * c
                eng = nc.sync if c == NCH - 1 else nc.gpsimd
                eng.dma_start(
                    out=out_d[q0:q0 + CHW, :].rearrange("(j p) e -> p j e", p=PB),
                    in_=o_sb[:].rearrange("p (j e) -> p j e", e=E))
    nc.finalize()
    return nc


def make_core_inputs(key_np, value_np, query_np, Wk, Wv, Wq):
    """Host-side sharding: returns in_maps list of 8 dicts."""
    bf = lambda a: np.ascontiguousarray(a).astype(NPBF16)
    # permute K/V columns within each 512-chunk: [j0 j2 j1 j3]
    perm = np.concatenate([512 * kc + np.concatenate(
        [np.arange(128 * j, 128 * j + 128) for j in KVPERM])
        for kc in range(S // 512)])
    ki = np.arange(PB)[:, None]
    qi = np.arange(PB)[None, :]
    tri = (ki <= qi).astype(np.float32)
    ones = np.ones((PB, PB), np.float32)
    zeros = np.zeros((PB, PB), np.float32)
    in_maps = []
    for c in range(8):
        b, h = c // 2, c % 2
        qrows = np.concatenate(
            [np.arange(PB * (2 * j + h), PB * (2 * j + h) + PB) for j in range(NLQ)])
        # mask8[m]: multiply for the [128] section at C0TAB[m] of new-kb m
        mask8 = np.zeros((8, PB, PB), dtype=np.float32)
        for m in range(8):
            if m % 2 == 0:
                mask8[m] = tri if h == 0 else ones
            else:
                mask8[m] = zeros if h == 0 else tri
        id2 = np.zeros((PB, E), np.float32)
        id2[E:2 * E, :] = np.eye(E)
        in_maps.append({
            "xq": bf(query_np[b][qrows].T),
            "xk": bf(key_np[b].T[:, perm]),
            "xv": bf(value_np[b].T[:, perm]),
            "wq": bf(Wq), "wk": bf(Wk), "wv": bf(Wv),
            "mask8": bf(mask8),
            "ident": np.eye(PB, dtype=np.float32),
            "ident2": bf(id2),
        })
    return in_maps


def assemble_output(results):
    """results: list of 8 dicts with 'out' [2048, 64] f32 -> Z [B,S,E]."""
    Z = np.zeros((B, S, E), dtype=np.float32)
    for c in range(8):
        b, h = c // 2, c % 2
        o = results[c]["out"]  # [2048, E] q-major
        for j in range(NLQ):
            g = 2 * j + h
            Z[b, PB * g:PB * (g + 1), :] = o[PB * j:PB * (j + 1), :]
    return Z


def kernel(key_inputs, value_inputs, query_inputs, Wk, Wv, Wq):
    from concourse.bass_utils import run_bass_kernel_spmd
    nc = build_nc()
    in_maps = make_core_inputs(np.asarray(key_inputs), np.asarray(value_inputs),
                               np.asarray(query_inputs), np.asarray(Wk),
                               np.asarray(Wv), np.asarray(Wq))
    res = run_bass_kernel_spmd(nc, in_maps, core_ids=list(range(8)))
    return assemble_output(res.results)
